# revision 16
# baseline (speedup 1.0000x reference)
"""Differentiable-stack kernel for Trainium2 (Bass/Tile), 8-core data parallel.

The reference soft stack only ever reads slot S-1, and the shift moves slot
s+1 -> slot s (never upward), so the output reduces to a gated linear
recurrence per (batch, d):

    y_t = a_t * y_{t-1} + b_t * x_t
    a_t = (1-p_t)(1-o_t),  b_t = p_t (1-o_t)      (scalars per (b, t))

Per core: 2 batch elements of [L=2048, D=512] f32.  The recurrence runs
chunk-wise on the TensorEngine as a triangular-band matmul plus an exact
rank-1 cross-chunk carry:

    y[s:s+T] = C_c^T . x[s:s+T]  +  P_c (x) y[s-1]
    C_c[j,t] = b_j * prod_{k=j+1..t} a_k = exp(S_t - S_j + ln b_j)  (j<=t)
    P_c[t]   = prod_{k=s..t} a_k = exp(S_t)     (S = in-chunk cumsum ln a)

C_c tiles are built 4 chunks at a time in PSUM with three matmuls
(S-row broadcast; j>t "ramp" suppression -1000*max(j-t,0); per-partition
bias ln b_j - S_j via a K=4 block-indicator matmul) and one ScalarE EXP.
Output rows are rotated by one (psum row p = timestep p-1) so the carry
row sits at partition 0; store DMAs un-rotate.  x is cast f32->fp16 by
SWDGE load DMAs; y is computed and stored in f32 via HWDGE (sync) DMAs.
"""

import os
from contextlib import ExitStack

import numpy as np

import concourse.bass as bass
import concourse.tile as tile
from concourse import bacc, mybir
from concourse.bass_utils import run_bass_kernel_spmd

F32 = mybir.dt.float32
F16 = mybir.dt.float16
ALU = mybir.AluOpType
ACTF = mybir.ActivationFunctionType

B, L, D = 16, 2048, 512
NCORES = 8
BPC = B // NCORES            # batches per core = 2
T = 128                      # chunk length
NC = L // T                  # chunks per batch element = 16
SEG = BPC * NC               # gate-tensor partitions = 32
G4 = 4                       # chunks per Ct-build group
NG = NC // G4                # Ct groups per batch = 4

LGROUPS = [int(g) for g in os.environ.get("DSTACK_LG", "2,4,5,5").split(",")]
SGROUPS = [int(g) for g in os.environ.get("DSTACK_SG", "2,2,4,4,4").split(",")]
PSYC = int(os.environ.get("DSTACK_PSYC", "2"))     # chunks per psum group
PSYBUFS = int(os.environ.get("DSTACK_PSY", "3"))
CTBUFS = int(os.environ.get("DSTACK_CT", "3"))
DVE_FRAC = int(os.environ.get("DSTACK_DVECOLS", "256"))  # DVE cols per 512

assert sum(LGROUPS) == NC and sum(SGROUPS) == NC


def build_module():
    nc = bacc.Bacc("TRN2", target_bir_lowering=False)
    xin = nc.dram_tensor("xin", [T, BPC * NC * D], F32, kind="ExternalInput")
    pg = nc.dram_tensor("pg", [SEG, T], F32, kind="ExternalInput")
    og = nc.dram_tensor("og", [SEG, T], F32, kind="ExternalInput")
    yout = nc.dram_tensor("yout", [T, BPC * NC * D], F32, kind="ExternalOutput")
    # DRAM bounce buffers -> partition-0 row layouts for matmul operands
    scr_s = nc.dram_tensor("scr_s", [1, SEG * T], F32, kind="Internal")
    scr_p = nc.dram_tensor("scr_p", [1, SEG * T], F16, kind="Internal")
    scr_b = nc.dram_tensor("scr_b", [1, SEG * T], F32, kind="Internal")

    with tile.TileContext(nc) as tc, ExitStack() as ctx:
        smalls = ctx.enter_context(tc.tile_pool(name="smalls", bufs=1))
        xpool = ctx.enter_context(tc.tile_pool(name="xpool", bufs=1))
        ypool = ctx.enter_context(tc.tile_pool(name="ypool", bufs=1))
        ctpool = ctx.enter_context(tc.tile_pool(name="ctpool", bufs=CTBUFS))
        carrypool = ctx.enter_context(tc.tile_pool(name="carrypool", bufs=2))
        pspool = ctx.enter_context(tc.tile_pool(name="pspool", bufs=1,
                                                space="PSUM"))

        # -------- gate DMAs (HWDGE sync queue) -----------------------------
        pgt = smalls.tile([SEG, T], F32)
        ogt = smalls.tile([SEG, T], F32)
        nc.sync.dma_start(pgt[:], pg[:])
        nc.sync.dma_start(ogt[:], og[:])

        # -------- x cast-loads (SWDGE, f32 HBM -> fp16 SBUF), few and big --
        xtiles = {}          # (b, c) -> (tile, col0)
        for b in range(BPC):
            c0 = 0
            for gi, g in enumerate(LGROUPS):
                xt = xpool.tile([T, g * D], F16, tag=f"x{b}_{gi}")
                for c in range(c0, c0 + g):
                    xtiles[(b, c)] = (xt, (c - c0) * D)
                c0 += g
        for gi, g in enumerate(LGROUPS):
            c0 = sum(LGROUPS[:gi])
            for b in range(BPC):
                xt = xtiles[(b, c0)][0]
                lo = (b * NC + c0) * D
                nc.gpsimd.dma_start(xt[:], xin[:, lo:lo + g * D])

        # -------- constants (gpsimd Q7, after load descriptor emission) ----
        # Output-row rotation: psum row p holds t'=p-1; row 0 holds t'=127.
        ident32 = smalls.tile([32, 32], F32)
        nc.gpsimd.memset(ident32[:], 0.0)
        nc.gpsimd.affine_select(
            out=ident32[:], in_=ident32[:], compare_op=ALU.not_equal, fill=1.0,
            base=0, pattern=[[-1, 32]], channel_multiplier=1)
        # ramp: psum += sum_k L[k,j]*U4[k,(q,p)] = -1000*max(j - t'(p), 0)
        lmat = smalls.tile([T, T], F16)
        nc.gpsimd.memset(lmat[:], 1.0)
        nc.gpsimd.affine_select(
            out=lmat[:], in_=lmat[:], compare_op=ALU.is_ge, fill=0.0,
            base=-1, pattern=[[1, T]], channel_multiplier=-1)
        umat4 = smalls.tile([T, G4, T], F16)
        nc.gpsimd.memset(umat4[:], -1000.0)
        nc.gpsimd.affine_select(
            out=umat4[:], in_=umat4[:], compare_op=ALU.is_ge, fill=0.0,
            base=1, pattern=[[0, G4], [-1, T]], channel_multiplier=1)
        nc.gpsimd.affine_select(
            out=umat4[:], in_=umat4[:], compare_op=ALU.is_ge, fill=0.0,
            base=-1, pattern=[[0, G4], [1, T]], channel_multiplier=0)
        # block indicator: blk[k, (q, t')] = 1{k == q}  (bias spread matmul)
        blk = smalls.tile([G4, G4, T], F32)
        nc.gpsimd.memset(blk[:], 0.0)
        nc.gpsimd.affine_select(
            out=blk[:], in_=blk[:], compare_op=ALU.not_equal, fill=1.0,
            base=0, pattern=[[-1, G4], [0, T]], channel_multiplier=1)

        # -------- gate math (tiny, [SEG, T]) -------------------------------
        ones_st = smalls.tile([SEG, T], F32)
        nc.vector.memset(ones_st[:], 1.0)
        ones_row = smalls.tile([1, T], F32)
        nc.vector.memset(ones_row[:], 1.0)

        om = smalls.tile([SEG, T], F32)
        av = smalls.tile([SEG, T], F32)
        bv = smalls.tile([SEG, T], F32)
        nc.vector.tensor_scalar(om[:], ogt[:], -1.0, 1.0, ALU.mult, ALU.add)
        nc.vector.tensor_scalar(av[:], pgt[:], -1.0, 1.0, ALU.mult, ALU.add)
        nc.vector.tensor_mul(av[:], av[:], om[:])
        nc.vector.tensor_mul(bv[:], pgt[:], om[:])

        la = smalls.tile([SEG, T], F32)
        nc.scalar.activation(la[:], av[:], ACTF.Ln)
        lb = smalls.tile([SEG, T], F32)
        nc.scalar.activation(lb[:], bv[:], ACTF.Ln)

        sv = smalls.tile([SEG, T], F32)
        nc.vector.tensor_tensor_scan(sv[:], ones_st[:], la[:], 0.0,
                                     ALU.mult, ALU.add)
        bias = smalls.tile([SEG, T], F32)
        nc.vector.tensor_sub(bias[:], lb[:], sv[:])
        prow = smalls.tile([SEG, T], F16)
        nc.scalar.activation(prow[:], sv[:], ACTF.Exp)

        # bounce rotated S/P rows and 4-seg-grouped bias through DRAM
        srows = smalls.tile([1, SEG * T], F32)
        prows = smalls.tile([1, SEG * T], F16)
        biasg = smalls.tile([G4, 2 * NG, T], F32)
        scr_s2 = scr_s[:].rearrange("o (p f) -> (o p) f", f=T)
        scr_p2 = scr_p[:].rearrange("o (p f) -> (o p) f", f=T)
        nc.sync.dma_start(scr_s2[:, 0:1], sv[:, T - 1:T])
        nc.sync.dma_start(scr_s2[:, 1:T], sv[:, 0:T - 1])
        nc.sync.dma_start(scr_p2[:, 0:1], prow[:, T - 1:T])
        nc.sync.dma_start(scr_p2[:, 1:T], prow[:, 0:T - 1])
        nc.sync.dma_start(scr_b[:].rearrange("o (p f) -> (o p) f", f=T),
                          bias[:])
        nc.sync.dma_start(srows[:], scr_s[:])
        nc.sync.dma_start(prows[:], scr_p[:])
        nc.sync.dma_start(
            biasg[:], scr_b[:].rearrange("o (g p f) -> (o p) g f",
                                         p=G4, f=T))

        # -------- store plan -----------------------------------------------
        sgrp = {}
        for b in range(BPC):
            c0 = 0
            for gi, g in enumerate(SGROUPS):
                yt = ypool.tile([T, g * D], F32, tag=f"y{b}_{gi}")
                for c in range(c0, c0 + g):
                    sgrp[(b, c)] = (yt, (c - c0) * D, c == c0 + g - 1,
                                    (b * NC + c0) * D, g)
                c0 += g

        # -------- main loop ------------------------------------------------
        cts = {}             # (b, group) -> ct tile [T, G4*T] fp16
        carries = {}
        psys = {}
        for c in range(NC):
            for b in range(BPC):
                seg = b * NC + c
                q = c // G4
                if c % G4 == 0:
                    # build Ct for chunks [c, c+4): bcast + ramp + bias, exp
                    ps2 = pspool.tile([T, G4 * T], F32, tag="p2", bufs=2)
                    nc.tensor.matmul(ps2[:], ones_row[:],
                                     srows[0:1, seg * T:(seg + G4) * T],
                                     start=True, stop=False)
                    nc.tensor.matmul(
                        ps2[:], biasg[:, b * NG + q, :],
                        blk[:].rearrange("p a b -> p (a b)"),
                        start=False, stop=False, skip_group_check=True)
                    nc.tensor.matmul(ps2[:], lmat[:],
                                     umat4[:].rearrange("p a b -> p (a b)"),
                                     start=False, stop=True,
                                     skip_group_check=True)
                    ct = ctpool.tile([T, G4 * T], F16, tag=f"ct{b}")
                    nc.scalar.activation(ct[:], ps2[:], ACTF.Exp)
                    cts[(b, q)] = ct

                ct = cts[(b, q)]
                xt, xcol = xtiles[(b, c)]
                if c % PSYC == 0:
                    psy = pspool.tile([T, PSYC * D], F32, tag="psy",
                                      bufs=PSYBUFS, name=f"psy{b}_{c}")
                    psys[b] = psy
                psy = psys[b]
                pcol = (c % PSYC) * D
                first = (c == 0)
                nc.tensor.matmul(psy[:, pcol:pcol + D],
                                 ct[:, (c % G4) * T:(c % G4 + 1) * T],
                                 xt[:, xcol:xcol + D],
                                 start=True, stop=first)
                if not first:
                    nc.tensor.matmul(psy[:, pcol:pcol + D],
                                     prows[0:1, seg * T:(seg + 1) * T],
                                     carries[b][:], start=False, stop=True,
                                     skip_group_check=True)

                # carry for next chunk: rotated rows put t'=127 at partition 0
                if c < NC - 1:
                    cw = carrypool.tile([1, D], F16, tag=f"cw{b}")
                    if c % 2 == 0:
                        nc.scalar.copy(cw[:], psy[0:1, pcol:pcol + D])
                    else:
                        nc.vector.tensor_copy(cw[:], psy[0:1, pcol:pcol + D])
                    carries[b] = cw

                # psum group -> sbuf y (split ScalarE / DVE), once per group
                if c % PSYC == PSYC - 1 or c == NC - 1:
                    yt, ycol, _, _, _ = sgrp[(b, c - (c % PSYC))]
                    w = (c % PSYC + 1) * D
                    nds = DVE_FRAC * w // 512
                    nc.scalar.copy(yt[:, ycol:ycol + w - nds],
                                   psy[:, 0:w - nds])
                    if nds:
                        nc.vector.tensor_copy(yt[:, ycol + w - nds:ycol + w],
                                              psy[:, w - nds:w])
                yt, ycol, last, dcol0, g = sgrp[(b, c)]
                if last:
                    # un-rotate at store time (row p holds t'=p-1)
                    nc.sync.dma_start(yout[0:T - 1, dcol0:dcol0 + g * D],
                                      yt[1:T, :])
                    nc.sync.dma_start(yout[T - 1:T, dcol0:dcol0 + g * D],
                                      yt[0:1, :])

    nc.compile()
    return nc


_module_cache = {}


def _get_module():
    if "nc" not in _module_cache:
        _module_cache["nc"] = build_module()
    return _module_cache["nc"]


def make_in_maps(x, push_gate, pop_gate):
    x = np.ascontiguousarray(np.asarray(x), dtype=np.float32)
    pgf = np.asarray(push_gate, dtype=np.float32).reshape(B, L)
    ogf = np.asarray(pop_gate, dtype=np.float32).reshape(B, L)
    in_maps = []
    for i in range(NCORES):
        sl = slice(i * BPC, (i + 1) * BPC)
        xi = x[sl].reshape(BPC, NC, T, D).transpose(2, 0, 1, 3)
        in_maps.append({
            "xin": np.ascontiguousarray(xi.reshape(T, BPC * NC * D)),
            "pg": np.ascontiguousarray(pgf[sl].reshape(SEG, T)),
            "og": np.ascontiguousarray(ogf[sl].reshape(SEG, T)),
        })
    return in_maps


def run(x, push_gate, pop_gate, **spmd_kwargs):
    """Run on hardware; returns (output, BassKernelResults)."""
    nc = _get_module()
    in_maps = make_in_maps(x, push_gate, pop_gate)
    res = run_bass_kernel_spmd(nc, in_maps, core_ids=list(range(NCORES)),
                               **spmd_kwargs)
    outs = []
    for i in range(NCORES):
        yo = res.results[i]["yout"].reshape(T, BPC, NC, D)
        outs.append(yo.transpose(1, 2, 0, 3).reshape(BPC, L, D))
    return np.concatenate(outs, axis=0), res


def kernel(x, push_gate, pop_gate):
    out, _ = run(x, push_gate, pop_gate)
    return out


# revision 20
# speedup vs baseline: 3.2627x; 3.2627x over previous
"""Differentiable-stack kernel for Trainium2 (Bass/Tile), 8-core data parallel.

The reference soft stack only ever reads slot S-1, and the shift moves slot
s+1 -> slot s (never upward), so the output reduces to a gated linear
recurrence per (batch, d):

    y_t = a_t * y_{t-1} + b_t * x_t
    a_t = (1-p_t)(1-o_t),  b_t = p_t (1-o_t)      (scalars per (b, t))

Per core: 2 batch elements of [L=2048, D=512] f32.  The recurrence runs
chunk-wise on the TensorEngine as a triangular-band matmul plus an exact
rank-1 cross-chunk carry:

    y[s:s+T] = C_c^T . x[s:s+T]  +  P_c (x) y[s-1]
    C_c[j,t] = b_j * prod_{k=j+1..t} a_k = exp(S_t - S_j + ln b_j)  (j<=t)
    P_c[t]   = prod_{k=s..t} a_k = exp(S_t)     (S = in-chunk cumsum ln a)

C_c tiles are built 4 chunks at a time in PSUM with three matmuls
(S-row broadcast; j>t "ramp" suppression -1000*max(j-t,0); per-partition
bias ln b_j - S_j via a K=4 block-indicator matmul) and one ScalarE EXP.
Output rows are rotated by one (psum row p = timestep p-1) so the carry
row sits at partition 0; store DMAs un-rotate.  x is cast f32->fp16 by
SWDGE load DMAs; y is computed and stored in f32 via HWDGE (sync) DMAs.
"""

import os
from contextlib import ExitStack

import numpy as np

import concourse.bass as bass
import concourse.tile as tile
from concourse import bacc, mybir
from concourse.bass_utils import run_bass_kernel_spmd

F32 = mybir.dt.float32
F16 = mybir.dt.float16
ALU = mybir.AluOpType
ACTF = mybir.ActivationFunctionType

B, L, D = 16, 2048, 512
NCORES = 8
BPC = B // NCORES            # batches per core = 2
T = 128                      # chunk length
NC = L // T                  # chunks per batch element = 16
SEG = BPC * NC               # gate-tensor partitions = 32
G4 = 4                       # chunks per Ct-build group
NG = NC // G4                # Ct groups per batch = 4

LGROUPS = [int(g) for g in os.environ.get("DSTACK_LG", "2,4,5,5").split(",")]
SGROUPS = [int(g) for g in os.environ.get("DSTACK_SG", "2,2,4,4,4").split(",")]
PSYC = int(os.environ.get("DSTACK_PSYC", "2"))     # chunks per psum group
PSYBUFS = int(os.environ.get("DSTACK_PSY", "3"))
CTBUFS = int(os.environ.get("DSTACK_CT", "3"))
DVE_FRAC = int(os.environ.get("DSTACK_DVECOLS", "256"))  # DVE cols per 512

assert sum(LGROUPS) == NC and sum(SGROUPS) == NC


def build_module():
    nc = bacc.Bacc("TRN2", target_bir_lowering=False)
    xin = nc.dram_tensor("xin", [T, BPC * NC * D], F32, kind="ExternalInput")
    pg = nc.dram_tensor("pg", [SEG, T], F32, kind="ExternalInput")
    og = nc.dram_tensor("og", [SEG, T], F32, kind="ExternalInput")
    yout = nc.dram_tensor("yout", [T, BPC * NC * D], F32, kind="ExternalOutput")
    # DRAM bounce buffers -> partition-0 row layouts for matmul operands
    scr_s = nc.dram_tensor("scr_s", [1, SEG * T], F32, kind="Internal")
    scr_p = nc.dram_tensor("scr_p", [1, SEG * T], F16, kind="Internal")
    scr_b = nc.dram_tensor("scr_b", [1, SEG * T], F32, kind="Internal")

    with tile.TileContext(nc) as tc, ExitStack() as ctx:
        smalls = ctx.enter_context(tc.tile_pool(name="smalls", bufs=1))
        xpool = ctx.enter_context(tc.tile_pool(name="xpool", bufs=1))
        ypool = ctx.enter_context(tc.tile_pool(name="ypool", bufs=1))
        ctpool = ctx.enter_context(tc.tile_pool(name="ctpool", bufs=CTBUFS))
        carrypool = ctx.enter_context(tc.tile_pool(name="carrypool", bufs=2))
        pspool = ctx.enter_context(tc.tile_pool(name="pspool", bufs=1,
                                                space="PSUM"))

        # -------- gate DMAs (HWDGE sync queue) -----------------------------
        pgt = smalls.tile([SEG, T], F32)
        ogt = smalls.tile([SEG, T], F32)
        nc.sync.dma_start(pgt[:], pg[:])
        nc.sync.dma_start(ogt[:], og[:])

        # -------- x loads (HWDGE f32), few and big; engines cast to fp16 ---
        xtiles = {}          # (b, c) -> (fp16 tile, col0)
        xf32 = {}
        for b in range(BPC):
            c0 = 0
            for gi, g in enumerate(LGROUPS):
                xt = xpool.tile([T, g * D], F16, tag=f"x{b}_{gi}")
                xf = xpool.tile([T, g * D], F32, tag=f"xf{b}_{gi}")
                xf32[(b, gi)] = (xf, g, sum(LGROUPS[:gi]))
                for c in range(c0, c0 + g):
                    xtiles[(b, c)] = (xt, (c - c0) * D)
                c0 += g
        for gi, g in enumerate(LGROUPS):
            c0 = sum(LGROUPS[:gi])
            for b in range(BPC):
                xf = xf32[(b, gi)][0]
                lo = (b * NC + c0) * D
                nc.sync.dma_start(xf[:], xin[:, lo:lo + g * D])

        # -------- constants (gpsimd Q7, after load descriptor emission) ----
        # Output-row rotation: psum row p holds t'=p-1; row 0 holds t'=127.
        ident32 = smalls.tile([32, 32], F32)
        nc.gpsimd.memset(ident32[:], 0.0)
        nc.gpsimd.affine_select(
            out=ident32[:], in_=ident32[:], compare_op=ALU.not_equal, fill=1.0,
            base=0, pattern=[[-1, 32]], channel_multiplier=1)
        # ramp: psum += sum_k L[k,j]*U4[k,(q,p)] = -1000*max(j - t'(p), 0)
        lmat = smalls.tile([T, T], F16)
        nc.gpsimd.memset(lmat[:], 1.0)
        nc.gpsimd.affine_select(
            out=lmat[:], in_=lmat[:], compare_op=ALU.is_ge, fill=0.0,
            base=-1, pattern=[[1, T]], channel_multiplier=-1)
        umat4 = smalls.tile([T, G4, T], F16)
        nc.gpsimd.memset(umat4[:], -1000.0)
        nc.gpsimd.affine_select(
            out=umat4[:], in_=umat4[:], compare_op=ALU.is_ge, fill=0.0,
            base=1, pattern=[[0, G4], [-1, T]], channel_multiplier=1)
        nc.gpsimd.affine_select(
            out=umat4[:], in_=umat4[:], compare_op=ALU.is_ge, fill=0.0,
            base=-1, pattern=[[0, G4], [1, T]], channel_multiplier=0)
        # block indicator: blk[k, (q, t')] = 1{k == q}  (bias spread matmul)
        blk = smalls.tile([G4, G4, T], F32)
        nc.gpsimd.memset(blk[:], 0.0)
        nc.gpsimd.affine_select(
            out=blk[:], in_=blk[:], compare_op=ALU.not_equal, fill=1.0,
            base=0, pattern=[[-1, G4], [0, T]], channel_multiplier=1)

        # -------- gate math (tiny, [SEG, T]) -------------------------------
        ones_st = smalls.tile([SEG, T], F32)
        nc.vector.memset(ones_st[:], 1.0)
        ones_row = smalls.tile([1, T], F32)
        nc.vector.memset(ones_row[:], 1.0)

        om = smalls.tile([SEG, T], F32)
        av = smalls.tile([SEG, T], F32)
        bv = smalls.tile([SEG, T], F32)
        nc.vector.tensor_scalar(om[:], ogt[:], -1.0, 1.0, ALU.mult, ALU.add)
        nc.vector.tensor_scalar(av[:], pgt[:], -1.0, 1.0, ALU.mult, ALU.add)
        nc.vector.tensor_mul(av[:], av[:], om[:])
        nc.vector.tensor_mul(bv[:], pgt[:], om[:])

        la = smalls.tile([SEG, T], F32)
        nc.scalar.activation(la[:], av[:], ACTF.Ln)
        lb = smalls.tile([SEG, T], F32)
        nc.scalar.activation(lb[:], bv[:], ACTF.Ln)

        sv = smalls.tile([SEG, T], F32)
        nc.vector.tensor_tensor_scan(sv[:], ones_st[:], la[:], 0.0,
                                     ALU.mult, ALU.add)
        bias = smalls.tile([SEG, T], F32)
        nc.vector.tensor_sub(bias[:], lb[:], sv[:])
        prow = smalls.tile([SEG, T], F16)
        nc.scalar.activation(prow[:], sv[:], ACTF.Exp)

        # bounce rotated S/P rows and 4-seg-grouped bias through DRAM
        srows = smalls.tile([1, SEG * T], F32)
        prows = smalls.tile([1, SEG * T], F16)
        biasg = smalls.tile([G4, 2 * NG, T], F32)
        scr_s2 = scr_s[:].rearrange("o (p f) -> (o p) f", f=T)
        scr_p2 = scr_p[:].rearrange("o (p f) -> (o p) f", f=T)
        nc.sync.dma_start(scr_s2[:, 0:1], sv[:, T - 1:T])
        nc.sync.dma_start(scr_s2[:, 1:T], sv[:, 0:T - 1])
        nc.sync.dma_start(scr_p2[:, 0:1], prow[:, T - 1:T])
        nc.sync.dma_start(scr_p2[:, 1:T], prow[:, 0:T - 1])
        nc.sync.dma_start(scr_b[:].rearrange("o (p f) -> (o p) f", f=T),
                          bias[:])
        nc.sync.dma_start(srows[:], scr_s[:])
        nc.sync.dma_start(prows[:], scr_p[:])
        nc.sync.dma_start(
            biasg[:], scr_b[:].rearrange("o (g p f) -> (o p) g f",
                                         p=G4, f=T))

        # -------- x casts f32 -> fp16 (DVE / gpsimd halves) ----------------
        for gi, g in enumerate(LGROUPS):
            for b in range(BPC):
                xf = xf32[(b, gi)][0]
                xt = xtiles[(b, sum(LGROUPS[:gi]))][0]
                h = g * D // 2
                nc.vector.tensor_copy(xt[:, 0:h], xf[:, 0:h])
                nc.gpsimd.tensor_copy(xt[:, h:g * D], xf[:, h:g * D])

        # -------- store plan -----------------------------------------------
        sgrp = {}
        for b in range(BPC):
            c0 = 0
            for gi, g in enumerate(SGROUPS):
                yt = ypool.tile([T, g * D], F32, tag=f"y{b}_{gi}")
                for c in range(c0, c0 + g):
                    sgrp[(b, c)] = (yt, (c - c0) * D, c == c0 + g - 1,
                                    (b * NC + c0) * D, g)
                c0 += g

        # -------- main loop ------------------------------------------------
        cts = {}             # (b, group) -> ct tile [T, G4*T] fp16
        carries = {}
        psys = {}
        for c in range(NC):
            for b in range(BPC):
                seg = b * NC + c
                q = c // G4
                if c % G4 == 0:
                    # build Ct for chunks [c, c+4): bcast + ramp + bias, exp
                    ps2 = pspool.tile([T, G4 * T], F32, tag="p2", bufs=2)
                    nc.tensor.matmul(ps2[:], ones_row[:],
                                     srows[0:1, seg * T:(seg + G4) * T],
                                     start=True, stop=False)
                    nc.tensor.matmul(
                        ps2[:], biasg[:, b * NG + q, :],
                        blk[:].rearrange("p a b -> p (a b)"),
                        start=False, stop=False, skip_group_check=True)
                    nc.tensor.matmul(ps2[:], lmat[:],
                                     umat4[:].rearrange("p a b -> p (a b)"),
                                     start=False, stop=True,
                                     skip_group_check=True)
                    ct = ctpool.tile([T, G4 * T], F16, tag=f"ct{b}")
                    nc.scalar.activation(ct[:], ps2[:], ACTF.Exp)
                    cts[(b, q)] = ct

                ct = cts[(b, q)]
                xt, xcol = xtiles[(b, c)]
                if c % PSYC == 0:
                    psy = pspool.tile([T, PSYC * D], F32, tag="psy",
                                      bufs=PSYBUFS, name=f"psy{b}_{c}")
                    psys[b] = psy
                psy = psys[b]
                pcol = (c % PSYC) * D
                first = (c == 0)
                nc.tensor.matmul(psy[:, pcol:pcol + D],
                                 ct[:, (c % G4) * T:(c % G4 + 1) * T],
                                 xt[:, xcol:xcol + D],
                                 start=True, stop=first)
                if not first:
                    nc.tensor.matmul(psy[:, pcol:pcol + D],
                                     prows[0:1, seg * T:(seg + 1) * T],
                                     carries[b][:], start=False, stop=True,
                                     skip_group_check=True)

                # carry for next chunk: rotated rows put t'=127 at partition 0
                if c < NC - 1:
                    cw = carrypool.tile([1, D], F16, tag=f"cw{b}")
                    if c % 2 == 0:
                        nc.scalar.copy(cw[:], psy[0:1, pcol:pcol + D])
                    else:
                        nc.vector.tensor_copy(cw[:], psy[0:1, pcol:pcol + D])
                    carries[b] = cw

                # psum group -> sbuf y (split ScalarE / DVE), once per group
                if c % PSYC == PSYC - 1 or c == NC - 1:
                    yt, ycol, _, _, _ = sgrp[(b, c - (c % PSYC))]
                    w = (c % PSYC + 1) * D
                    nds = DVE_FRAC * w // 512
                    nc.scalar.copy(yt[:, ycol:ycol + w - nds],
                                   psy[:, 0:w - nds])
                    if nds:
                        nc.vector.tensor_copy(yt[:, ycol + w - nds:ycol + w],
                                              psy[:, w - nds:w])
                yt, ycol, last, dcol0, g = sgrp[(b, c)]
                if last:
                    # store rotated (row p holds t'=p-1); host un-rotates
                    nc.sync.dma_start(yout[:, dcol0:dcol0 + g * D], yt[:])

    nc.compile()
    return nc


_module_cache = {}


def _get_module():
    if "nc" not in _module_cache:
        _module_cache["nc"] = build_module()
    return _module_cache["nc"]


def make_in_maps(x, push_gate, pop_gate):
    x = np.ascontiguousarray(np.asarray(x), dtype=np.float32)
    pgf = np.asarray(push_gate, dtype=np.float32).reshape(B, L)
    ogf = np.asarray(pop_gate, dtype=np.float32).reshape(B, L)
    in_maps = []
    for i in range(NCORES):
        sl = slice(i * BPC, (i + 1) * BPC)
        xi = x[sl].reshape(BPC, NC, T, D).transpose(2, 0, 1, 3)
        in_maps.append({
            "xin": np.ascontiguousarray(xi.reshape(T, BPC * NC * D)),
            "pg": np.ascontiguousarray(pgf[sl].reshape(SEG, T)),
            "og": np.ascontiguousarray(ogf[sl].reshape(SEG, T)),
        })
    return in_maps


def run(x, push_gate, pop_gate, **spmd_kwargs):
    """Run on hardware; returns (output, BassKernelResults)."""
    nc = _get_module()
    in_maps = make_in_maps(x, push_gate, pop_gate)
    res = run_bass_kernel_spmd(nc, in_maps, core_ids=list(range(NCORES)),
                               **spmd_kwargs)
    outs = []
    for i in range(NCORES):
        yo = res.results[i]["yout"].reshape(T, BPC, NC, D)
        yo = np.roll(yo, -1, axis=0)  # un-rotate: row p holds t'=p-1
        outs.append(yo.transpose(1, 2, 0, 3).reshape(BPC, L, D))
    return np.concatenate(outs, axis=0), res


def kernel(x, push_gate, pop_gate):
    out, _ = run(x, push_gate, pop_gate)
    return out
